# revision 46
# baseline (speedup 1.0000x reference)
"""Trainium2 Bass kernel for nn_MultiHeadAttention (B=2,T=2048,D=1024,H=16,HD=64).

Sharding: 8 cores = 2 batches x 4 heads/core (tensor parallel over heads).
Each core computes q,k,v projections for its 4 heads, RoPE, causal
flash-attention, and a partial output projection (its heads' slice of Wp);
the host sums the 4 partials per batch.

v2 design (single fully-pipelined pass, tensor-engine saturated):
  - One interleaved instruction stream: projection / output-projection
    matmul "quanta" are drained into the softmax-wait bubbles of the
    attention k-loop so the PE never idles (keeps the 2.4GHz p-state).
  - Softmax denominators come for free from augmented-V matmuls:
    lhsT = [v_h | ones] (even heads) / [ones | v_h] (odd heads), so each
    head's PV bank rows carry both the attention numerator and 64 copies
    of the denominator, partition-aligned with the pair layout that the
    output projection needs. No separate ones-matmuls, no zero-init
    matmuls (first PV uses start=True).
  - Scores computed transposed ([k, q]); causal mask applied by ONE extra
    accumulating matmul with constant ramp matrices (adds -1e4*max(0,k-q)).
  - exp on the scalar engine only (scale folded in, no max-subtraction:
    |s*scale| <= ~4); double-buffered score psum so exp pipelines.
  - RoPE split across vector (lo half) and gpsimd (hi half) engines.
  - asum psum drained to SBUF by one gpsimd copy so the single accumulator
    psum buffer recycles fast; reciprocal via one fast custom-DVE op.
  - PSUM: scores 2x2 banks, accumulator 2, proj/outproj ring 2 = 8 exact.
  - All dram inputs pre-cast to bf16 on host (same RTNE rounding as chip).
"""

import os
import sys

sys.path.insert(0, "/opt/trn_rl_repo")

from contextlib import ExitStack

import numpy as np
import ml_dtypes

import concourse.bass as bass
import concourse.bacc as bacc
import concourse.tile as tile
import concourse.mybir as mybir
from concourse.bass import ts, ds
from concourse.bass_utils import run_bass_kernel_spmd

B, T, D, H, HD = 2, 2048, 1024, 16, 64
HPC = 4                # heads per core
E = HPC * HD           # 256 per-core channels
WP = 512               # projection chunk width (t)
WA = 256               # attention chunk width (q)
NPC = T // WP          # 4
NAC = T // WA          # 8
NKT = T // 128         # 16 k-tiles
DQ = D // 128          # 8 contraction subtiles
NEG = -10000.0
FP32 = mybir.dt.float32
BF16 = mybir.dt.bfloat16
SCALE = 1.0 / float(np.sqrt(HD))
NTT = T // 128         # 16 t-tiles for the output projection


def build_program(level=99):
    # level: debug truncation. 0=setup, 1=+prologue proj, 2=+chunk0 attn,
    # 3=+chunks<=3 w/ proj quanta, 4=+all chunks, 99=full (outproj+tail)
    nc = bacc.Bacc("TRN2", target_bir_lowering=False, debug=False)
    xT_in = nc.declare_dram_parameter("xT_b", [D, T], BF16, isOutput=False)
    wqT = nc.declare_dram_parameter("wqT", [D, E], BF16, isOutput=False)
    wkT = nc.declare_dram_parameter("wkT", [D, E], BF16, isOutput=False)
    wvT = nc.declare_dram_parameter("wvT", [D, E], BF16, isOutput=False)
    wpT = nc.declare_dram_parameter("wpT", [E, D], BF16, isOutput=False)
    cosT = nc.declare_dram_parameter("cosT", [128, T], BF16, isOutput=False)
    sinT = nc.declare_dram_parameter("sinT", [128, T], BF16, isOutput=False)
    dmask = nc.declare_dram_parameter("dmask", [128, 2, WA], BF16, isOutput=False)
    outp = nc.declare_dram_parameter("outp", [T, D], FP32, isOutput=True)

    with tile.TileContext(nc) as tc, ExitStack() as ctx:
        consts = ctx.enter_context(tc.tile_pool(name="consts", bufs=1))
        ropet = ctx.enter_context(tc.tile_pool(name="ropet", bufs=2))
        probs_p = ctx.enter_context(
            tc.tile_pool(name="probs", bufs=int(os.environ.get("K_PRBUFS", "2")))
        )
        asb_p = ctx.enter_context(tc.tile_pool(name="asb", bufs=2))
        den_p = ctx.enter_context(tc.tile_pool(name="den", bufs=2))
        ostage = ctx.enter_context(tc.tile_pool(name="ostage", bufs=2))
        ps_sc = ctx.enter_context(
            tc.tile_pool(
                name="ps_sc", bufs=int(os.environ.get("K_SCBUFS", "2")), space="PSUM"
            )
        )
        ps_acc = ctx.enter_context(tc.tile_pool(name="ps_acc", bufs=1, space="PSUM"))
        ps_io = ctx.enter_context(tc.tile_pool(name="ps_io", bufs=1, space="PSUM"))

        # ---- constants / weights / x to SBUF ----
        # issue order matters: q-proj needs wq + xT first; cos/sin by rope
        # time; the diag mask only by the first attention chunk.
        xT_sb = consts.tile([128, DQ, T], BF16, tag="xT")
        xT_r = xT_in.rearrange("(o p) m -> p o m", p=128)
        wq_sb = consts.tile([128, DQ, E], BF16, tag="wq")
        nc.gpsimd.dma_start(wq_sb[:], wqT.rearrange("(o p) m -> p o m", p=128))
        for dq in range(DQ):
            eng = nc.gpsimd if dq % 2 == 0 else nc.sync
            eng.dma_start(xT_sb[:, dq, :], xT_r[:, dq, :])
        wk_sb = consts.tile([128, DQ, E], BF16, tag="wk")
        nc.gpsimd.dma_start(wk_sb[:], wkT.rearrange("(o p) m -> p o m", p=128))
        cos_sb = consts.tile([128, T], BF16, tag="cos")
        nc.scalar.dma_start(cos_sb[:], cosT[:])
        sin_sb = consts.tile([128, T], BF16, tag="sin")
        nc.scalar.dma_start(sin_sb[:], sinT[:])
        wv_sb = consts.tile([128, DQ, E], BF16, tag="wv")
        nc.gpsimd.dma_start(wv_sb[:], wvT.rearrange("(o p) m -> p o m", p=128))
        wp_sb = consts.tile([128, 2, D], BF16, tag="wp")
        nc.sync.dma_start(wp_sb[:], wpT.rearrange("(o p) m -> p o m", p=128))
        dm_sb = consts.tile([128, 2, WA], BF16, tag="dmask")
        nc.sync.dma_start(dm_sb[:], dmask[:])

        # per-head q/k tiles on partitions 0:64 — keeps every scores matmul
        # at PE tile_position (0,0); base-64 matmuls closing a group on a
        # partial psum bank crash the runtime.
        q_nat = [
            consts.tile([64, T], BF16, tag=f"qnat{h}", name=f"qnat{h}")
            for h in range(HPC)
        ]
        k_nat = [
            consts.tile([64, T], BF16, tag=f"knat{h}", name=f"knat{h}")
            for h in range(HPC)
        ]
        # v_aug[:, t, h, :]: even h = [v | ones], odd h = [ones | v]; fill
        # everything with ones, the v copies overwrite their halves.
        v_aug = consts.tile([128, NKT, HPC, 128], BF16, tag="vaug")
        nc.gpsimd.memset(v_aug[:], 1.0)
        attn_nrm = [
            consts.tile([128, T], BF16, tag=f"anrm{p}", name=f"anrm{p}")
            for p in range(2)
        ]
        zer_sb = consts.tile([128, 128], BF16, tag="zer")
        nc.gpsimd.memset(zer_sb[:], 0.0)

        # ---- work quanta (proj / outproj), drained between attn iters ----
        def emit_qk(j, w_sb, nat):
            pqk = ps_io.tile([128, 2, WP], FP32, tag="io", name="pqk")
            for half in range(2):
                for dq in range(DQ):
                    nc.tensor.matmul(
                        pqk[:, half, :],
                        lhsT=w_sb[:, dq, ds(128 * half, 128)],
                        rhs=xT_sb[:, dq, ts(j, WP)],
                        start=(dq == 0),
                        stop=(dq == DQ - 1),
                    )
            lo, hi = pqk[:, 0, :], pqk[:, 1, :]
            cs, sn = cos_sb[:, ts(j, WP)], sin_sb[:, ts(j, WP)]
            st = ropet.tile([128, 2, WP], BF16, tag="st", name="st")
            ta = ropet.tile([128, 2, WP], FP32, tag="ta", name="ta")
            tb = ropet.tile([128, 2, WP], FP32, tag="tb", name="tb")
            nc.vector.tensor_mul(ta[:, 0, :], lo, cs)
            nc.vector.tensor_mul(ta[:, 1, :], hi, sn)
            nc.vector.tensor_sub(st[:, 0, :], ta[:, 0, :], ta[:, 1, :])
            nc.vector.tensor_mul(tb[:, 0, :], hi, cs)
            nc.vector.tensor_mul(tb[:, 1, :], lo, sn)
            nc.vector.tensor_add(st[:, 1, :], tb[:, 0, :], tb[:, 1, :])
            for h in range(HPC):
                nc.sync.dma_start(
                    nat[h][ds(0, 32), ts(j, WP)], st[ds(32 * h, 32), 0, :]
                )
                nc.sync.dma_start(
                    nat[h][ds(32, 32), ts(j, WP)], st[ds(32 * h, 32), 1, :]
                )

        def emit_v(j, half_pair):
            pv = ps_io.tile([128, 2, E], FP32, tag="io", name="pv")
            for tt in range(2):
                g = 4 * j + 2 * half_pair + tt
                for dq in range(DQ):
                    nc.tensor.matmul(
                        pv[:, tt, :],
                        lhsT=xT_sb[:, dq, ts(g, 128)],
                        rhs=wv_sb[:, dq, :],
                        start=(dq == 0),
                        stop=(dq == DQ - 1),
                    )
            for tt in range(2):
                g = 4 * j + 2 * half_pair + tt
                for h in range(HPC):
                    voff = 0 if h % 2 == 0 else 64
                    nc.vector.tensor_copy(
                        v_aug[:, g, h, ds(voff, 64)], pv[:, tt, ds(64 * h, 64)]
                    )

        def emit_po(g):
            po = ps_io.tile([128, D], FP32, tag="io", name="po")
            for dh in range(2):
                for p in range(2):
                    nc.tensor.matmul(
                        po[:, ds(512 * dh, 512)],
                        lhsT=attn_nrm[p][:, ts(g, 128)],
                        rhs=wp_sb[:, p, ds(512 * dh, 512)],
                        start=(p == 0),
                        stop=(p == 1),
                    )
            ost = ostage.tile([128, D], FP32, tag="ost", name="ost")
            nc.vector.tensor_copy(ost[:], po[:])
            if g % 2 == 0:
                nc.gpsimd.dma_start(outp[ts(g, 128), :], ost[:])
            else:
                nc.sync.dma_start(outp[ts(g, 128), :], ost[:])

        pending = []
        gap = [0]

        def drain_one():
            if pending and gap[0] >= 2:
                pending.pop(0)()
                gap[0] = 0

        # prologue: projection chunk 0 (serial; nothing to overlap with yet)
        if level >= 1 and not os.environ.get("K_NOPRO"):
            emit_qk(0, wq_sb, q_nat)
            emit_qk(0, wk_sb, k_nat)
            emit_v(0, 0)
            emit_v(0, 1)

        n_chunks = {0: 0, 1: 0, 2: 1, 3: 4, 4: NAC}.get(level, NAC)
        if os.environ.get("K_NCHUNKS"):
            n_chunks = int(os.environ["K_NCHUNKS"])
        a_start = int(os.environ.get("K_CHUNK_START", "0"))
        # ---- attention chunks, with quanta interleaved ----
        for a in range(a_start, n_chunks):
            if a % 2 == 0 and level >= 3 and not os.environ.get("K_NOQUANTA"):
                j = a // 2 + 1
                if j < NPC:
                    pending.append(lambda j=j: emit_qk(j, wq_sb, q_nat))
                    pending.append(lambda j=j: emit_qk(j, wk_sb, k_nat))
                    pending.append(lambda j=j: emit_v(j, 0))
                    pending.append(lambda j=j: emit_v(j, 1))
            if a >= 5 and level >= 5:
                # outproj for t-tiles, deferred late to fill bare iters
                po_sched = {5: range(0, 6), 6: range(6, 12), 7: range(12, 14)}
                for g in po_sched[a]:
                    pending.append(lambda g=g: emit_po(g))

            nk = 2 * a + 2
            if os.environ.get("K_MAXNK"):
                nk = min(nk, int(os.environ["K_MAXNK"]))
            asum = None
            if not os.environ.get("K_NOPV"):
                asum = ps_acc.tile([128, HPC, WA], FP32, tag="acc", name="asum")
            # start=True zeroes whole 2KB psum banks, so a per-head start
            # would wipe the co-banked head's accumulation; zero-init each
            # bank with one full-bank matmul instead.
            for bank in range(0 if not os.environ.get("K_NOPV") else -1, 2 if not os.environ.get("K_NOPV") else -1):
                nc.tensor.matmul(
                    asum[:, ds(2 * bank, 2), :],
                    lhsT=zer_sb[:],
                    rhs=xT_sb[:, 0, 0:512],
                    start=True,
                    stop=False,
                    skip_group_check=True,
                )

            def S(i, a=a):
                sct = ps_sc.tile([128, HPC, WA], FP32, tag="sc", name="sct")
                for h in range(HPC):
                    nc.tensor.matmul(
                        sct[:, h, :],
                        lhsT=k_nat[h][:, ts(i, 128)],
                        rhs=q_nat[h][:, ts(a, WA)],
                        start=True,
                        stop=True,
                    )
                return sct

            def EPV(i, sct, nk=nk, asum=asum, a=a):
                pr = probs_p.tile([128, HPC, WA], BF16, tag="pr", name="pr")
                nc.scalar.activation(
                    pr[:], sct[:], mybir.ActivationFunctionType.Exp, scale=SCALE
                )
                if i >= 2 * a:
                    # causal mask: zero the upper-triangular probs of the
                    # (at most two) diagonal k-tiles
                    d = i - 2 * a
                    for h in range(HPC):
                        nc.gpsimd.tensor_mul(
                            pr[:, h, :], pr[:, h, :], dm_sb[:, d, :]
                        )
                if os.environ.get("K_NOPV"):
                    return
                for h in range(HPC):
                    nc.tensor.matmul(
                        asum[:, h, :],
                        lhsT=v_aug[:, i, h, :],
                        rhs=pr[:, h, :],
                        start=False,
                        stop=(i == nk - 1),
                        skip_group_check=True,
                    )

            prev = None
            for i in range(nk):
                sct = S(i)
                if prev is not None:
                    EPV(prev[0], prev[1])
                prev = (i, sct)
                gap[0] += 1
                drain_one()
            EPV(prev[0], prev[1])

            if os.environ.get("K_NOEPI"):
                continue
            # epilogue: drain asum fast, then normalize off the critical path
            asb = asb_p.tile([128, HPC, WA], FP32, tag="asb", name="asb")
            nc.vector.tensor_copy(asb[:], asum[:])
            den = den_p.tile([128, 2, WA], FP32, tag="den", name="den")
            for p in range(2):
                nc.sync.dma_start(den[ds(0, 64), p, :], asb[ds(64, 64), 2 * p, :])
                nc.sync.dma_start(den[ds(64, 64), p, :], asb[ds(0, 64), 2 * p + 1, :])
            rc = den_p.tile([128, 2, WA], FP32, tag="rc", name="rc")
            nc.vector.reciprocal_approx_fast(rc[:], den[:])
            for p in range(2):
                nc.gpsimd.tensor_mul(
                    attn_nrm[p][ds(0, 64), ts(a, WA)],
                    asb[ds(0, 64), 2 * p, :],
                    rc[ds(0, 64), p, :],
                )
                nc.gpsimd.tensor_mul(
                    attn_nrm[p][ds(64, 64), ts(a, WA)],
                    asb[ds(64, 64), 2 * p + 1, :],
                    rc[ds(64, 64), p, :],
                )

        # tail: whatever quanta remain + last output tiles
        for f in pending:
            f()
        if level >= 5:
            for g in range(NTT - 2, NTT):
                emit_po(g)

    nc.compile()
    return nc


def make_consts(cos, sin):
    cosT = np.ascontiguousarray(
        np.tile(np.asarray(cos[0], dtype=np.float32).T[:32], (4, 1))
    ).astype(ml_dtypes.bfloat16)
    sinT = np.ascontiguousarray(
        np.tile(np.asarray(sin[0], dtype=np.float32).T[:32], (4, 1))
    ).astype(ml_dtypes.bfloat16)
    k = np.arange(128)[:, None]
    q = np.arange(WA)[None, :]
    dmask = np.stack([(q >= k), (q >= 128 + k)], axis=1).astype(ml_dtypes.bfloat16)
    return dict(cosT=cosT, sinT=sinT, dmask=dmask)


def host_prep(core, xT_by_batch, Wq, Wk, Wv, Wp, consts):
    b, hp = core // 4, core % 4
    h0 = hp * HPC
    rows = slice(HD * h0, HD * h0 + E)
    Wq_s = np.asarray(Wq[rows]).reshape(HPC, HD, D)
    Wk_s = np.asarray(Wk[rows]).reshape(HPC, HD, D)
    wqT = np.ascontiguousarray(
        np.concatenate(
            [Wq_s[:, :32].reshape(128, D), Wq_s[:, 32:].reshape(128, D)], 0
        ).T.astype(ml_dtypes.bfloat16)
    )
    wkT = np.ascontiguousarray(
        np.concatenate(
            [Wk_s[:, :32].reshape(128, D), Wk_s[:, 32:].reshape(128, D)], 0
        ).T.astype(ml_dtypes.bfloat16)
    )
    wvT = np.ascontiguousarray(np.asarray(Wv[rows]).T.astype(ml_dtypes.bfloat16))
    wpT = np.ascontiguousarray(np.asarray(Wp[:, rows]).T.astype(ml_dtypes.bfloat16))
    return dict(
        xT_b=xT_by_batch[b],
        wqT=wqT,
        wkT=wkT,
        wvT=wvT,
        wpT=wpT,
        **consts,
    )


_NC_CACHE = None


def _get_nc():
    global _NC_CACHE
    if _NC_CACHE is None:
        _NC_CACHE = build_program()
    return _NC_CACHE


def kernel(x, cos, sin, Wq, Wk, Wv, Wp, _want_trace=False):
    x, cos, sin = np.asarray(x), np.asarray(cos), np.asarray(sin)
    Wq, Wk, Wv, Wp = (np.asarray(a) for a in (Wq, Wk, Wv, Wp))
    nc = _get_nc()
    consts = make_consts(cos, sin)
    xT_by_batch = [
        np.ascontiguousarray(x[b].T.astype(ml_dtypes.bfloat16)) for b in range(B)
    ]
    in_maps = [
        host_prep(core, xT_by_batch, Wq, Wk, Wv, Wp, consts) for core in range(8)
    ]
    res = run_bass_kernel_spmd(nc, in_maps, list(range(8)), trace=_want_trace)
    out = np.zeros((B, T, D), dtype=np.float32)
    for core in range(8):
        out[core // 4] += np.asarray(res.results[core]["outp"], dtype=np.float32)
    if _want_trace:
        kernel.last_exec_time_ns = res.exec_time_ns
        kernel.last_profile = res.profile_json
    return out


# revision 51
# speedup vs baseline: 1.0510x; 1.0510x over previous
"""Trainium2 Bass kernel for nn_MultiHeadAttention (B=2,T=2048,D=1024,H=16,HD=64).

Sharding: 8 cores = 2 batches x 4 heads/core (tensor parallel over heads).
Each core computes q,k,v projections for its 4 heads, RoPE, causal
flash-attention, and a partial output projection (its heads' slice of Wp);
the host sums the 4 partials per batch.

v2 design (single fully-pipelined pass, tensor-engine saturated):
  - One interleaved instruction stream: projection / output-projection
    matmul "quanta" are drained into the softmax-wait bubbles of the
    attention k-loop so the PE never idles (keeps the 2.4GHz p-state).
  - Softmax denominators come for free from augmented-V matmuls:
    lhsT = [v_h | ones] (even heads) / [ones | v_h] (odd heads), so each
    head's PV bank rows carry both the attention numerator and 64 copies
    of the denominator, partition-aligned with the pair layout that the
    output projection needs. No separate ones-matmuls, no zero-init
    matmuls (first PV uses start=True).
  - Scores computed transposed ([k, q]); causal mask applied by ONE extra
    accumulating matmul with constant ramp matrices (adds -1e4*max(0,k-q)).
  - exp on the scalar engine only (scale folded in, no max-subtraction:
    |s*scale| <= ~4); double-buffered score psum so exp pipelines.
  - RoPE split across vector (lo half) and gpsimd (hi half) engines.
  - asum psum drained to SBUF by one gpsimd copy so the single accumulator
    psum buffer recycles fast; reciprocal via one fast custom-DVE op.
  - PSUM: scores 2x2 banks, accumulator 2, proj/outproj ring 2 = 8 exact.
  - All dram inputs pre-cast to bf16 on host (same RTNE rounding as chip).
"""

import os
import sys

sys.path.insert(0, "/opt/trn_rl_repo")

from contextlib import ExitStack

import numpy as np
import ml_dtypes

import concourse.bass as bass
import concourse.bacc as bacc
import concourse.tile as tile
import concourse.mybir as mybir
from concourse.bass import ts, ds
from concourse.bass_utils import run_bass_kernel_spmd

B, T, D, H, HD = 2, 2048, 1024, 16, 64
HPC = 4                # heads per core
E = HPC * HD           # 256 per-core channels
WP = 512               # projection chunk width (t)
WA = 256               # attention chunk width (q)
NPC = T // WP          # 4
NAC = T // WA          # 8
NKT = T // 128         # 16 k-tiles
DQ = D // 128          # 8 contraction subtiles
NEG = -10000.0
FP32 = mybir.dt.float32
BF16 = mybir.dt.bfloat16
SCALE = 1.0 / float(np.sqrt(HD))
NTT = T // 128         # 16 t-tiles for the output projection


def build_program(level=99):
    # level: debug truncation. 0=setup, 1=+prologue proj, 2=+chunk0 attn,
    # 3=+chunks<=3 w/ proj quanta, 4=+all chunks, 99=full (outproj+tail)
    nc = bacc.Bacc("TRN2", target_bir_lowering=False, debug=False)
    xT_in = nc.declare_dram_parameter("xT_b", [D, T], BF16, isOutput=False)
    wqT = nc.declare_dram_parameter("wqT", [D, E], BF16, isOutput=False)
    wkT = nc.declare_dram_parameter("wkT", [D, E], BF16, isOutput=False)
    wvT = nc.declare_dram_parameter("wvT", [D, E], BF16, isOutput=False)
    wpT = nc.declare_dram_parameter("wpT", [E, D], BF16, isOutput=False)
    cosT = nc.declare_dram_parameter("cosT", [128, T], BF16, isOutput=False)
    sinT = nc.declare_dram_parameter("sinT", [128, T], BF16, isOutput=False)
    umask = nc.declare_dram_parameter("umask", [128, 128], BF16, isOutput=False)
    lmask = nc.declare_dram_parameter("lmask", [128, 640], BF16, isOutput=False)
    outp = nc.declare_dram_parameter("outp", [T, D], FP32, isOutput=True)

    with tile.TileContext(nc) as tc, ExitStack() as ctx:
        consts = ctx.enter_context(tc.tile_pool(name="consts", bufs=1))
        ropet = ctx.enter_context(tc.tile_pool(name="ropet", bufs=2))
        probs_p = ctx.enter_context(
            tc.tile_pool(name="probs", bufs=int(os.environ.get("K_PRBUFS", "2")))
        )
        asb_p = ctx.enter_context(tc.tile_pool(name="asb", bufs=2))
        den_p = ctx.enter_context(tc.tile_pool(name="den", bufs=2))
        ostage = ctx.enter_context(tc.tile_pool(name="ostage", bufs=2))
        ps_sc = ctx.enter_context(
            tc.tile_pool(
                name="ps_sc", bufs=int(os.environ.get("K_SCBUFS", "2")), space="PSUM"
            )
        )
        ps_acc = ctx.enter_context(tc.tile_pool(name="ps_acc", bufs=1, space="PSUM"))
        ps_io = ctx.enter_context(tc.tile_pool(name="ps_io", bufs=1, space="PSUM"))

        # ---- constants / weights / x to SBUF ----
        # issue order matters: q-proj needs wq + xT first; cos/sin by rope
        # time; the diag mask only by the first attention chunk.
        xT_sb = consts.tile([128, DQ, T], BF16, tag="xT")
        xT_r = xT_in.rearrange("(o p) m -> p o m", p=128)
        wq_sb = consts.tile([128, DQ, E], BF16, tag="wq")
        nc.gpsimd.dma_start(wq_sb[:], wqT.rearrange("(o p) m -> p o m", p=128))
        for dq in range(DQ):
            eng = nc.gpsimd if dq % 2 == 0 else nc.sync
            eng.dma_start(xT_sb[:, dq, :], xT_r[:, dq, :])
        wk_sb = consts.tile([128, DQ, E], BF16, tag="wk")
        nc.gpsimd.dma_start(wk_sb[:], wkT.rearrange("(o p) m -> p o m", p=128))
        cos_sb = consts.tile([128, T], BF16, tag="cos")
        nc.scalar.dma_start(cos_sb[:], cosT[:])
        sin_sb = consts.tile([128, T], BF16, tag="sin")
        nc.scalar.dma_start(sin_sb[:], sinT[:])
        wv_sb = consts.tile([128, DQ, E], BF16, tag="wv")
        nc.gpsimd.dma_start(wv_sb[:], wvT.rearrange("(o p) m -> p o m", p=128))
        wp_sb = consts.tile([128, 2, D], BF16, tag="wp")
        nc.sync.dma_start(wp_sb[:], wpT.rearrange("(o p) m -> p o m", p=128))
        u_sb = consts.tile([128, 128], BF16, tag="umask")
        nc.sync.dma_start(u_sb[:], umask[:])
        lm_sb = consts.tile([128, 640], BF16, tag="lmask")
        nc.sync.dma_start(lm_sb[:], lmask[:])

        # per-head q/k tiles on partitions 0:64 — keeps every scores matmul
        # at PE tile_position (0,0); base-64 matmuls closing a group on a
        # partial psum bank crash the runtime.
        q_nat = [
            consts.tile([64, T], BF16, tag=f"qnat{h}", name=f"qnat{h}")
            for h in range(HPC)
        ]
        k_nat = [
            consts.tile([64, T], BF16, tag=f"knat{h}", name=f"knat{h}")
            for h in range(HPC)
        ]
        # v_aug[:, t, h, :]: even h = [v | ones], odd h = [ones | v]; fill
        # everything with ones, the v copies overwrite their halves.
        v_aug = consts.tile([128, NKT, HPC, 128], BF16, tag="vaug")
        nc.gpsimd.memset(v_aug[:], 1.0)
        attn_nrm = [
            consts.tile([128, T], BF16, tag=f"anrm{p}", name=f"anrm{p}")
            for p in range(2)
        ]
        zer_sb = consts.tile([128, 128], BF16, tag="zer")
        nc.gpsimd.memset(zer_sb[:], 0.0)

        # ---- work quanta (proj / outproj), drained between attn iters ----
        def emit_qk(j, w_sb, nat):
            pqk = ps_io.tile([128, 2, WP], FP32, tag="io", name="pqk")
            for half in range(2):
                for dq in range(DQ):
                    nc.tensor.matmul(
                        pqk[:, half, :],
                        lhsT=w_sb[:, dq, ds(128 * half, 128)],
                        rhs=xT_sb[:, dq, ts(j, WP)],
                        start=(dq == 0),
                        stop=(dq == DQ - 1),
                    )
            lo, hi = pqk[:, 0, :], pqk[:, 1, :]
            cs, sn = cos_sb[:, ts(j, WP)], sin_sb[:, ts(j, WP)]
            st = ropet.tile([128, 2, WP], BF16, tag="st", name="st")
            ta = ropet.tile([128, 2, WP], FP32, tag="ta", name="ta")
            tb = ropet.tile([128, 2, WP], FP32, tag="tb", name="tb")
            nc.vector.tensor_mul(ta[:, 0, :], lo, cs)
            nc.vector.tensor_mul(ta[:, 1, :], hi, sn)
            nc.vector.tensor_sub(st[:, 0, :], ta[:, 0, :], ta[:, 1, :])
            nc.vector.tensor_mul(tb[:, 0, :], hi, cs)
            nc.vector.tensor_mul(tb[:, 1, :], lo, sn)
            nc.vector.tensor_add(st[:, 1, :], tb[:, 0, :], tb[:, 1, :])
            for h in range(HPC):
                nc.sync.dma_start(
                    nat[h][ds(0, 32), ts(j, WP)], st[ds(32 * h, 32), 0, :]
                )
                nc.sync.dma_start(
                    nat[h][ds(32, 32), ts(j, WP)], st[ds(32 * h, 32), 1, :]
                )

        def emit_v(j, half_pair):
            pv = ps_io.tile([128, 2, E], FP32, tag="io", name="pv")
            for tt in range(2):
                g = 4 * j + 2 * half_pair + tt
                for dq in range(DQ):
                    nc.tensor.matmul(
                        pv[:, tt, :],
                        lhsT=xT_sb[:, dq, ts(g, 128)],
                        rhs=wv_sb[:, dq, :],
                        start=(dq == 0),
                        stop=(dq == DQ - 1),
                    )
            for tt in range(2):
                g = 4 * j + 2 * half_pair + tt
                for h in range(HPC):
                    voff = 0 if h % 2 == 0 else 64
                    nc.vector.tensor_copy(
                        v_aug[:, g, h, ds(voff, 64)], pv[:, tt, ds(64 * h, 64)]
                    )

        def emit_po(g):
            po = ps_io.tile([128, D], FP32, tag="io", name="po")
            for dh in range(2):
                for p in range(2):
                    nc.tensor.matmul(
                        po[:, ds(512 * dh, 512)],
                        lhsT=attn_nrm[p][:, ts(g, 128)],
                        rhs=wp_sb[:, p, ds(512 * dh, 512)],
                        start=(p == 0),
                        stop=(p == 1),
                    )
            ost = ostage.tile([128, D], FP32, tag="ost", name="ost")
            nc.vector.tensor_copy(ost[:], po[:])
            if g % 2 == 0:
                nc.gpsimd.dma_start(outp[ts(g, 128), :], ost[:])
            else:
                nc.sync.dma_start(outp[ts(g, 128), :], ost[:])

        pending = []
        gap = [0]

        def drain_one():
            if pending and gap[0] >= 2:
                pending.pop(0)()
                gap[0] = 0

        # prologue: projection chunk 0 (serial; nothing to overlap with yet)
        if level >= 1 and not os.environ.get("K_NOPRO"):
            emit_qk(0, wq_sb, q_nat)
            emit_qk(0, wk_sb, k_nat)
            emit_v(0, 0)
            emit_v(0, 1)

        n_chunks = {0: 0, 1: 0, 2: 1, 3: 4, 4: NAC}.get(level, NAC)
        if os.environ.get("K_NCHUNKS"):
            n_chunks = int(os.environ["K_NCHUNKS"])
        a_start = int(os.environ.get("K_CHUNK_START", "0"))
        # ---- attention chunks, with quanta interleaved ----
        for a in range(a_start, n_chunks):
            if a % 2 == 0 and level >= 3 and not os.environ.get("K_NOQUANTA"):
                j = a // 2 + 1
                if j < NPC:
                    pending.append(lambda j=j: emit_qk(j, wq_sb, q_nat))
                    pending.append(lambda j=j: emit_qk(j, wk_sb, k_nat))
                    pending.append(lambda j=j: emit_v(j, 0))
                    pending.append(lambda j=j: emit_v(j, 1))
            if a >= 5 and level >= 5:
                # outproj for t-tiles, deferred late to fill bare iters
                po_sched = {5: range(0, 6), 6: range(6, 12), 7: range(12, 14)}
                for g in po_sched[a]:
                    pending.append(lambda g=g: emit_po(g))

            nk = 2 * a + 2
            if os.environ.get("K_MAXNK"):
                nk = min(nk, int(os.environ["K_MAXNK"]))
            asum = None
            if not os.environ.get("K_NOPV"):
                asum = ps_acc.tile([128, HPC, WA], FP32, tag="acc", name="asum")
            # start=True zeroes whole 2KB psum banks, so a per-head start
            # would wipe the co-banked head's accumulation; zero-init each
            # bank with one full-bank matmul instead.
            for bank in range(0 if not os.environ.get("K_NOPV") else -1, 2 if not os.environ.get("K_NOPV") else -1):
                nc.tensor.matmul(
                    asum[:, ds(2 * bank, 2), :],
                    lhsT=zer_sb[:],
                    rhs=xT_sb[:, 0, 0:512],
                    start=True,
                    stop=False,
                    skip_group_check=True,
                )

            def S(i, a=a):
                sct = ps_sc.tile([128, HPC, WA], FP32, tag="sc", name="sct")
                diag = i >= 2 * a
                for h in range(HPC):
                    nc.tensor.matmul(
                        sct[:, h, :],
                        lhsT=k_nat[h][:, ts(i, 128)],
                        rhs=q_nat[h][:, ts(a, WA)],
                        start=True,
                        stop=not diag,
                    )
                    if diag:
                        off = 384 - (128 * i - WA * a)
                        nc.tensor.matmul(
                            sct[:, h, :],
                            lhsT=u_sb[:],
                            rhs=lm_sb[:, ds(off, WA)],
                            start=False,
                            stop=True,
                        )
                return sct

            def EPV(i, sct, nk=nk, asum=asum, a=a):
                pr = probs_p.tile([128, HPC, WA], BF16, tag="pr", name="pr")
                nc.scalar.activation(
                    pr[:], sct[:], mybir.ActivationFunctionType.Exp, scale=SCALE
                )
                if os.environ.get("K_NOPV"):
                    return
                for h in range(HPC):
                    nc.tensor.matmul(
                        asum[:, h, :],
                        lhsT=v_aug[:, i, h, :],
                        rhs=pr[:, h, :],
                        start=False,
                        stop=(i == nk - 1),
                        skip_group_check=True,
                    )

            prev = None
            for i in range(nk):
                sct = S(i)
                if prev is not None:
                    EPV(prev[0], prev[1])
                prev = (i, sct)
                gap[0] += 1
                drain_one()
            EPV(prev[0], prev[1])

            if os.environ.get("K_NOEPI"):
                continue
            # epilogue: drain asum fast, then normalize off the critical path
            asb = asb_p.tile([128, HPC, WA], FP32, tag="asb", name="asb")
            nc.vector.tensor_copy(asb[:], asum[:])
            den = den_p.tile([128, 2, WA], FP32, tag="den", name="den")
            for p in range(2):
                nc.sync.dma_start(den[ds(0, 64), p, :], asb[ds(64, 64), 2 * p, :])
                nc.sync.dma_start(den[ds(64, 64), p, :], asb[ds(0, 64), 2 * p + 1, :])
            rc = den_p.tile([128, 2, WA], FP32, tag="rc", name="rc")
            nc.vector.reciprocal_approx_fast(rc[:], den[:])
            for p in range(2):
                nc.gpsimd.tensor_mul(
                    attn_nrm[p][ds(0, 64), ts(a, WA)],
                    asb[ds(0, 64), 2 * p, :],
                    rc[ds(0, 64), p, :],
                )
                nc.gpsimd.tensor_mul(
                    attn_nrm[p][ds(64, 64), ts(a, WA)],
                    asb[ds(64, 64), 2 * p + 1, :],
                    rc[ds(64, 64), p, :],
                )

        # tail: whatever quanta remain + last output tiles
        for f in pending:
            f()
        if level >= 5:
            for g in range(NTT - 2, NTT):
                emit_po(g)

    nc.compile()
    return nc


def make_consts(cos, sin):
    cosT = np.ascontiguousarray(
        np.tile(np.asarray(cos[0], dtype=np.float32).T[:32], (4, 1))
    ).astype(ml_dtypes.bfloat16)
    sinT = np.ascontiguousarray(
        np.tile(np.asarray(sin[0], dtype=np.float32).T[:32], (4, 1))
    ).astype(ml_dtypes.bfloat16)
    m = np.arange(128)[:, None]
    r = np.arange(128)[None, :]
    umask = np.where(r >= m, NEG, 0.0).astype(ml_dtypes.bfloat16)
    u_idx = np.arange(640)[None, :]
    lmask = (m >= u_idx - 383).astype(ml_dtypes.bfloat16)
    return dict(cosT=cosT, sinT=sinT, umask=umask, lmask=lmask)


def host_prep(core, xT_by_batch, Wq, Wk, Wv, Wp, consts):
    b, hp = core // 4, core % 4
    h0 = hp * HPC
    rows = slice(HD * h0, HD * h0 + E)
    Wq_s = np.asarray(Wq[rows]).reshape(HPC, HD, D)
    Wk_s = np.asarray(Wk[rows]).reshape(HPC, HD, D)
    wqT = np.ascontiguousarray(
        np.concatenate(
            [Wq_s[:, :32].reshape(128, D), Wq_s[:, 32:].reshape(128, D)], 0
        ).T.astype(ml_dtypes.bfloat16)
    )
    wkT = np.ascontiguousarray(
        np.concatenate(
            [Wk_s[:, :32].reshape(128, D), Wk_s[:, 32:].reshape(128, D)], 0
        ).T.astype(ml_dtypes.bfloat16)
    )
    wvT = np.ascontiguousarray(np.asarray(Wv[rows]).T.astype(ml_dtypes.bfloat16))
    wpT = np.ascontiguousarray(np.asarray(Wp[:, rows]).T.astype(ml_dtypes.bfloat16))
    return dict(
        xT_b=xT_by_batch[b],
        wqT=wqT,
        wkT=wkT,
        wvT=wvT,
        wpT=wpT,
        **consts,
    )


_NC_CACHE = None


def _get_nc():
    global _NC_CACHE
    if _NC_CACHE is None:
        _NC_CACHE = build_program()
    return _NC_CACHE


def kernel(x, cos, sin, Wq, Wk, Wv, Wp, _want_trace=False):
    x, cos, sin = np.asarray(x), np.asarray(cos), np.asarray(sin)
    Wq, Wk, Wv, Wp = (np.asarray(a) for a in (Wq, Wk, Wv, Wp))
    nc = _get_nc()
    consts = make_consts(cos, sin)
    xT_by_batch = [
        np.ascontiguousarray(x[b].T.astype(ml_dtypes.bfloat16)) for b in range(B)
    ]
    in_maps = [
        host_prep(core, xT_by_batch, Wq, Wk, Wv, Wp, consts) for core in range(8)
    ]
    res = run_bass_kernel_spmd(nc, in_maps, list(range(8)), trace=_want_trace)
    out = np.zeros((B, T, D), dtype=np.float32)
    for core in range(8):
        out[core // 4] += np.asarray(res.results[core]["outp"], dtype=np.float32)
    if _want_trace:
        kernel.last_exec_time_ns = res.exec_time_ns
        kernel.last_profile = res.profile_json
    return out


# revision 54
# speedup vs baseline: 1.0696x; 1.0177x over previous
"""Trainium2 Bass kernel for nn_MultiHeadAttention (B=2,T=2048,D=1024,H=16,HD=64).

Sharding: 8 cores = 2 batches x 4 heads/core (tensor parallel over heads).
Each core computes q,k,v projections for its 4 heads, RoPE, causal
flash-attention, and a partial output projection (its heads' slice of Wp);
the host sums the 4 partials per batch.

v2 design (single fully-pipelined pass, tensor-engine saturated):
  - One interleaved instruction stream: projection / output-projection
    matmul "quanta" are drained into the softmax-wait bubbles of the
    attention k-loop so the PE never idles (keeps the 2.4GHz p-state).
  - Softmax denominators come for free from augmented-V matmuls:
    lhsT = [v_h | ones] (even heads) / [ones | v_h] (odd heads), so each
    head's PV bank rows carry both the attention numerator and 64 copies
    of the denominator, partition-aligned with the pair layout that the
    output projection needs. No separate ones-matmuls, no zero-init
    matmuls (first PV uses start=True).
  - Scores computed transposed ([k, q]); causal mask applied by ONE extra
    accumulating matmul with constant ramp matrices (adds -1e4*max(0,k-q)).
  - exp on the scalar engine only (scale folded in, no max-subtraction:
    |s*scale| <= ~4); double-buffered score psum so exp pipelines.
  - RoPE split across vector (lo half) and gpsimd (hi half) engines.
  - asum psum drained to SBUF by one gpsimd copy so the single accumulator
    psum buffer recycles fast; reciprocal via one fast custom-DVE op.
  - PSUM: scores 2x2 banks, accumulator 2, proj/outproj ring 2 = 8 exact.
  - All dram inputs pre-cast to bf16 on host (same RTNE rounding as chip).
"""

import os
import sys

sys.path.insert(0, "/opt/trn_rl_repo")

from contextlib import ExitStack

import numpy as np
import ml_dtypes

import concourse.bass as bass
import concourse.bacc as bacc
import concourse.tile as tile
import concourse.mybir as mybir
from concourse.bass import ts, ds
from concourse.bass_utils import run_bass_kernel_spmd

B, T, D, H, HD = 2, 2048, 1024, 16, 64
HPC = 4                # heads per core
E = HPC * HD           # 256 per-core channels
WP = 512               # projection chunk width (t)
WA = 256               # attention chunk width (q)
NPC = T // WP          # 4
NAC = T // WA          # 8
NKT = T // 128         # 16 k-tiles
DQ = D // 128          # 8 contraction subtiles
NEG = -10000.0
FP32 = mybir.dt.float32
BF16 = mybir.dt.bfloat16
SCALE = 1.0 / float(np.sqrt(HD))
NTT = T // 128         # 16 t-tiles for the output projection


def build_program(level=99):
    # level: debug truncation. 0=setup, 1=+prologue proj, 2=+chunk0 attn,
    # 3=+chunks<=3 w/ proj quanta, 4=+all chunks, 99=full (outproj+tail)
    nc = bacc.Bacc("TRN2", target_bir_lowering=False, debug=False)
    xT_in = nc.declare_dram_parameter("xT_b", [D, T], BF16, isOutput=False)
    wqT = nc.declare_dram_parameter("wqT", [D, E], BF16, isOutput=False)
    wkT = nc.declare_dram_parameter("wkT", [D, E], BF16, isOutput=False)
    wvT = nc.declare_dram_parameter("wvT", [D, E], BF16, isOutput=False)
    wpT = nc.declare_dram_parameter("wpT", [E, D], BF16, isOutput=False)
    cosT = nc.declare_dram_parameter("cosT", [128, T], BF16, isOutput=False)
    sinT = nc.declare_dram_parameter("sinT", [128, T], BF16, isOutput=False)
    umask = nc.declare_dram_parameter("umask", [128, 128], BF16, isOutput=False)
    lmask = nc.declare_dram_parameter("lmask", [128, 640], BF16, isOutput=False)
    outp = nc.declare_dram_parameter("outp", [T, D], FP32, isOutput=True)

    with tile.TileContext(nc) as tc, ExitStack() as ctx:
        consts = ctx.enter_context(tc.tile_pool(name="consts", bufs=1))
        ropet = ctx.enter_context(tc.tile_pool(name="ropet", bufs=2))
        probs_p = ctx.enter_context(
            tc.tile_pool(name="probs", bufs=int(os.environ.get("K_PRBUFS", "2")))
        )
        asb_p = ctx.enter_context(tc.tile_pool(name="asb", bufs=2))
        den_p = ctx.enter_context(tc.tile_pool(name="den", bufs=2))
        ostage = ctx.enter_context(tc.tile_pool(name="ostage", bufs=2))
        ps_sc = ctx.enter_context(
            tc.tile_pool(
                name="ps_sc", bufs=int(os.environ.get("K_SCBUFS", "2")), space="PSUM"
            )
        )
        ps_acc = ctx.enter_context(tc.tile_pool(name="ps_acc", bufs=1, space="PSUM"))
        ps_io = ctx.enter_context(tc.tile_pool(name="ps_io", bufs=1, space="PSUM"))

        # ---- constants / weights / x to SBUF ----
        # issue order matters: q-proj needs wq + xT first; cos/sin by rope
        # time; the diag mask only by the first attention chunk.
        xT_sb = consts.tile([128, DQ, T], BF16, tag="xT")
        xT_r = xT_in.rearrange("(o p) m -> p o m", p=128)
        wq_sb = consts.tile([128, DQ, E], BF16, tag="wq")
        nc.gpsimd.dma_start(wq_sb[:], wqT.rearrange("(o p) m -> p o m", p=128))
        for dq in range(DQ):
            eng = nc.gpsimd if dq % 2 == 0 else nc.sync
            eng.dma_start(xT_sb[:, dq, :], xT_r[:, dq, :])
        wk_sb = consts.tile([128, DQ, E], BF16, tag="wk")
        nc.gpsimd.dma_start(wk_sb[:], wkT.rearrange("(o p) m -> p o m", p=128))
        cos_sb = consts.tile([128, T], BF16, tag="cos")
        nc.scalar.dma_start(cos_sb[:], cosT[:])
        sin_sb = consts.tile([128, T], BF16, tag="sin")
        nc.scalar.dma_start(sin_sb[:], sinT[:])
        wv_sb = consts.tile([128, DQ, E], BF16, tag="wv")
        nc.gpsimd.dma_start(wv_sb[:], wvT.rearrange("(o p) m -> p o m", p=128))
        wp_sb = consts.tile([128, 2, D], BF16, tag="wp")
        nc.sync.dma_start(wp_sb[:], wpT.rearrange("(o p) m -> p o m", p=128))
        u_sb = consts.tile([128, 128], BF16, tag="umask")
        nc.sync.dma_start(u_sb[:], umask[:])
        lm_sb = consts.tile([128, 640], BF16, tag="lmask")
        nc.sync.dma_start(lm_sb[:], lmask[:])

        # per-head q/k tiles on partitions 0:64 — keeps every scores matmul
        # at PE tile_position (0,0); base-64 matmuls closing a group on a
        # partial psum bank crash the runtime.
        q_nat = [
            consts.tile([64, T], BF16, tag=f"qnat{h}", name=f"qnat{h}")
            for h in range(HPC)
        ]
        k_nat = [
            consts.tile([64, T], BF16, tag=f"knat{h}", name=f"knat{h}")
            for h in range(HPC)
        ]
        # v_aug[:, t, h, :]: even h = [v | ones], odd h = [ones | v]; fill
        # everything with ones, the v copies overwrite their halves.
        v_aug = consts.tile([128, NKT, HPC, 128], BF16, tag="vaug")
        nc.gpsimd.memset(v_aug[:], 1.0)
        attn_nrm = [
            consts.tile([128, T], BF16, tag=f"anrm{p}", name=f"anrm{p}")
            for p in range(2)
        ]
        zer_sb = consts.tile([128, 128], BF16, tag="zer")
        nc.gpsimd.memset(zer_sb[:], 0.0)

        # ---- work quanta (proj / outproj), drained between attn iters ----
        def emit_qk(j, w_sb, nat):
            pqk = ps_io.tile([128, 2, WP], FP32, tag="io", name="pqk")
            for half in range(2):
                for dq in range(DQ):
                    nc.tensor.matmul(
                        pqk[:, half, :],
                        lhsT=w_sb[:, dq, ds(128 * half, 128)],
                        rhs=xT_sb[:, dq, ts(j, WP)],
                        start=(dq == 0),
                        stop=(dq == DQ - 1),
                    )
            lo, hi = pqk[:, 0, :], pqk[:, 1, :]
            cs, sn = cos_sb[:, ts(j, WP)], sin_sb[:, ts(j, WP)]
            st = ropet.tile([128, 2, WP], BF16, tag="st", name="st")
            ta = ropet.tile([128, 2, WP], FP32, tag="ta", name="ta")
            tb = ropet.tile([128, 2, WP], FP32, tag="tb", name="tb")
            nc.vector.tensor_mul(ta[:, 0, :], lo, cs)
            nc.vector.tensor_mul(ta[:, 1, :], hi, sn)
            nc.vector.tensor_sub(st[:, 0, :], ta[:, 0, :], ta[:, 1, :])
            nc.vector.tensor_mul(tb[:, 0, :], hi, cs)
            nc.vector.tensor_mul(tb[:, 1, :], lo, sn)
            nc.vector.tensor_add(st[:, 1, :], tb[:, 0, :], tb[:, 1, :])
            for h in range(HPC):
                nc.sync.dma_start(
                    nat[h][ds(0, 32), ts(j, WP)], st[ds(32 * h, 32), 0, :]
                )
                nc.sync.dma_start(
                    nat[h][ds(32, 32), ts(j, WP)], st[ds(32 * h, 32), 1, :]
                )

        def emit_v(j, half_pair):
            pv = ps_io.tile([128, 2, E], FP32, tag="io", name="pv")
            for tt in range(2):
                g = 4 * j + 2 * half_pair + tt
                for dq in range(DQ):
                    nc.tensor.matmul(
                        pv[:, tt, :],
                        lhsT=xT_sb[:, dq, ts(g, 128)],
                        rhs=wv_sb[:, dq, :],
                        start=(dq == 0),
                        stop=(dq == DQ - 1),
                    )
            for tt in range(2):
                g = 4 * j + 2 * half_pair + tt
                for h in range(HPC):
                    voff = 0 if h % 2 == 0 else 64
                    nc.vector.tensor_copy(
                        v_aug[:, g, h, ds(voff, 64)], pv[:, tt, ds(64 * h, 64)]
                    )

        def emit_po(g, pool=None, tag="io", tail=False):
            po = (pool or ps_io).tile([128, D], FP32, tag=tag, name="po")
            for dh in range(2):
                for p in range(2):
                    nc.tensor.matmul(
                        po[:, ds(512 * dh, 512)],
                        lhsT=attn_nrm[p][:, ts(g, 128)],
                        rhs=wp_sb[:, p, ds(512 * dh, 512)],
                        start=(p == 0),
                        stop=(p == 1),
                    )
            ost = ostage.tile([128, D], FP32, tag="ost", name="ost")
            if tail and g % 2 == 1:
                nc.scalar.copy(ost[:], po[:])
            else:
                nc.vector.tensor_copy(ost[:], po[:])
            if g % 2 == 0:
                nc.gpsimd.dma_start(outp[ts(g, 128), :], ost[:])
            else:
                nc.sync.dma_start(outp[ts(g, 128), :], ost[:])

        pending = []
        gap = [0]

        def drain_one():
            if pending and gap[0] >= 2:
                pending.pop(0)()
                gap[0] = 0

        # prologue: projection chunk 0 (serial; nothing to overlap with yet)
        if level >= 1 and not os.environ.get("K_NOPRO"):
            emit_qk(0, wq_sb, q_nat)
            emit_qk(0, wk_sb, k_nat)
            emit_v(0, 0)
            emit_v(0, 1)

        n_chunks = {0: 0, 1: 0, 2: 1, 3: 4, 4: NAC}.get(level, NAC)
        if os.environ.get("K_NCHUNKS"):
            n_chunks = int(os.environ["K_NCHUNKS"])
        a_start = int(os.environ.get("K_CHUNK_START", "0"))
        # ---- attention chunks, with quanta interleaved ----
        for a in range(a_start, n_chunks):
            if a % 2 == 0 and level >= 3 and not os.environ.get("K_NOQUANTA"):
                j = a // 2 + 1
                if j < NPC:
                    pending.append(lambda j=j: emit_qk(j, wq_sb, q_nat))
                    pending.append(lambda j=j: emit_qk(j, wk_sb, k_nat))
                    pending.append(lambda j=j: emit_v(j, 0))
                    pending.append(lambda j=j: emit_v(j, 1))
            if a >= 5 and level >= 5:
                # outproj for t-tiles, deferred late to fill bare iters
                po_sched = {5: range(0, 6), 6: range(6, 12), 7: range(12, 14)}
                for g in po_sched[a]:
                    pending.append(lambda g=g: emit_po(g))

            nk = 2 * a + 2
            if os.environ.get("K_MAXNK"):
                nk = min(nk, int(os.environ["K_MAXNK"]))
            asum = None
            if not os.environ.get("K_NOPV"):
                asum = ps_acc.tile([128, HPC, WA], FP32, tag="acc", name="asum")
            # start=True zeroes whole 2KB psum banks, so a per-head start
            # would wipe the co-banked head's accumulation; zero-init each
            # bank with one full-bank matmul instead.
            for bank in range(0 if not os.environ.get("K_NOPV") else -1, 2 if not os.environ.get("K_NOPV") else -1):
                nc.tensor.matmul(
                    asum[:, ds(2 * bank, 2), :],
                    lhsT=zer_sb[:],
                    rhs=xT_sb[:, 0, 0:512],
                    start=True,
                    stop=False,
                    skip_group_check=True,
                )

            def S(i, a=a):
                sct = ps_sc.tile([128, HPC, WA], FP32, tag="sc", name="sct")
                diag = i >= 2 * a
                for h in range(HPC):
                    nc.tensor.matmul(
                        sct[:, h, :],
                        lhsT=k_nat[h][:, ts(i, 128)],
                        rhs=q_nat[h][:, ts(a, WA)],
                        start=True,
                        stop=not diag,
                    )
                    if diag:
                        off = 384 - (128 * i - WA * a)
                        nc.tensor.matmul(
                            sct[:, h, :],
                            lhsT=u_sb[:],
                            rhs=lm_sb[:, ds(off, WA)],
                            start=False,
                            stop=True,
                        )
                return sct

            def EPV(i, sct, nk=nk, asum=asum, a=a):
                pr = probs_p.tile([128, HPC, WA], BF16, tag="pr", name="pr")
                nc.scalar.activation(
                    pr[:], sct[:], mybir.ActivationFunctionType.Exp, scale=SCALE
                )
                if os.environ.get("K_NOPV"):
                    return
                for h in range(HPC):
                    nc.tensor.matmul(
                        asum[:, h, :],
                        lhsT=v_aug[:, i, h, :],
                        rhs=pr[:, h, :],
                        start=False,
                        stop=(i == nk - 1),
                        skip_group_check=True,
                    )

            prev = None
            for i in range(nk):
                sct = S(i)
                if prev is not None:
                    EPV(prev[0], prev[1])
                prev = (i, sct)
                gap[0] += 1
                drain_one()
            EPV(prev[0], prev[1])

            if os.environ.get("K_NOEPI"):
                continue
            # epilogue: drain asum fast, then normalize off the critical path
            asb = asb_p.tile([128, HPC, WA], FP32, tag="asb", name="asb")
            nc.vector.tensor_copy(asb[:], asum[:])
            den = den_p.tile([128, 2, WA], FP32, tag="den", name="den")
            for p in range(2):
                nc.sync.dma_start(den[ds(0, 64), p, :], asb[ds(64, 64), 2 * p, :])
                nc.sync.dma_start(den[ds(64, 64), p, :], asb[ds(0, 64), 2 * p + 1, :])
            rc = den_p.tile([128, 2, WA], FP32, tag="rc", name="rc")
            nc.vector.reciprocal_approx_fast(rc[:], den[:])
            for p in range(2):
                nc.vector.tensor_mul(
                    attn_nrm[p][ds(0, 64), ts(a, WA)],
                    asb[ds(0, 64), 2 * p, :],
                    rc[ds(0, 64), p, :],
                )
                nc.vector.tensor_mul(
                    attn_nrm[p][ds(64, 64), ts(a, WA)],
                    asb[ds(64, 64), 2 * p + 1, :],
                    rc[ds(64, 64), p, :],
                )

        # tail: whatever quanta remain + last output tiles
        for f in pending:
            f()
        if level >= 5:
            # tail tiles go in the now-idle scores pool so they run in
            # parallel instead of serializing on the single-buffer io ring
            for g in range(NTT - 2, NTT):
                emit_po(g, pool=ps_sc, tag="sc", tail=True)

    nc.compile()
    return nc


def make_consts(cos, sin):
    cosT = np.ascontiguousarray(
        np.tile(np.asarray(cos[0], dtype=np.float32).T[:32], (4, 1))
    ).astype(ml_dtypes.bfloat16)
    sinT = np.ascontiguousarray(
        np.tile(np.asarray(sin[0], dtype=np.float32).T[:32], (4, 1))
    ).astype(ml_dtypes.bfloat16)
    m = np.arange(128)[:, None]
    r = np.arange(128)[None, :]
    umask = np.where(r >= m, NEG, 0.0).astype(ml_dtypes.bfloat16)
    u_idx = np.arange(640)[None, :]
    lmask = (m >= u_idx - 383).astype(ml_dtypes.bfloat16)
    return dict(cosT=cosT, sinT=sinT, umask=umask, lmask=lmask)


def host_prep(core, xT_by_batch, Wq, Wk, Wv, Wp, consts):
    b, hp = core // 4, core % 4
    h0 = hp * HPC
    rows = slice(HD * h0, HD * h0 + E)
    Wq_s = np.asarray(Wq[rows]).reshape(HPC, HD, D)
    Wk_s = np.asarray(Wk[rows]).reshape(HPC, HD, D)
    wqT = np.ascontiguousarray(
        np.concatenate(
            [Wq_s[:, :32].reshape(128, D), Wq_s[:, 32:].reshape(128, D)], 0
        ).T.astype(ml_dtypes.bfloat16)
    )
    wkT = np.ascontiguousarray(
        np.concatenate(
            [Wk_s[:, :32].reshape(128, D), Wk_s[:, 32:].reshape(128, D)], 0
        ).T.astype(ml_dtypes.bfloat16)
    )
    wvT = np.ascontiguousarray(np.asarray(Wv[rows]).T.astype(ml_dtypes.bfloat16))
    wpT = np.ascontiguousarray(np.asarray(Wp[:, rows]).T.astype(ml_dtypes.bfloat16))
    return dict(
        xT_b=xT_by_batch[b],
        wqT=wqT,
        wkT=wkT,
        wvT=wvT,
        wpT=wpT,
        **consts,
    )


_NC_CACHE = None


def _get_nc():
    global _NC_CACHE
    if _NC_CACHE is None:
        _NC_CACHE = build_program()
    return _NC_CACHE


def kernel(x, cos, sin, Wq, Wk, Wv, Wp, _want_trace=False):
    x, cos, sin = np.asarray(x), np.asarray(cos), np.asarray(sin)
    Wq, Wk, Wv, Wp = (np.asarray(a) for a in (Wq, Wk, Wv, Wp))
    nc = _get_nc()
    consts = make_consts(cos, sin)
    xT_by_batch = [
        np.ascontiguousarray(x[b].T.astype(ml_dtypes.bfloat16)) for b in range(B)
    ]
    in_maps = [
        host_prep(core, xT_by_batch, Wq, Wk, Wv, Wp, consts) for core in range(8)
    ]
    res = run_bass_kernel_spmd(nc, in_maps, list(range(8)), trace=_want_trace)
    out = np.zeros((B, T, D), dtype=np.float32)
    for core in range(8):
        out[core // 4] += np.asarray(res.results[core]["outp"], dtype=np.float32)
    if _want_trace:
        kernel.last_exec_time_ns = res.exec_time_ns
        kernel.last_profile = res.profile_json
    return out


# revision 55
# speedup vs baseline: 1.0860x; 1.0154x over previous
"""Trainium2 Bass kernel for nn_MultiHeadAttention (B=2,T=2048,D=1024,H=16,HD=64).

Sharding: 8 cores = 2 batches x 4 heads/core (tensor parallel over heads).
Each core computes q,k,v projections for its 4 heads, RoPE, causal
flash-attention, and a partial output projection (its heads' slice of Wp);
the host sums the 4 partials per batch.

v2 design (single fully-pipelined pass, tensor-engine saturated):
  - One interleaved instruction stream: projection / output-projection
    matmul "quanta" are drained into the softmax-wait bubbles of the
    attention k-loop so the PE never idles (keeps the 2.4GHz p-state).
  - Softmax denominators come for free from augmented-V matmuls:
    lhsT = [v_h | ones] (even heads) / [ones | v_h] (odd heads), so each
    head's PV bank rows carry both the attention numerator and 64 copies
    of the denominator, partition-aligned with the pair layout that the
    output projection needs. No separate ones-matmuls, no zero-init
    matmuls (first PV uses start=True).
  - Scores computed transposed ([k, q]); causal mask applied by ONE extra
    accumulating matmul with constant ramp matrices (adds -1e4*max(0,k-q)).
  - exp on the scalar engine only (scale folded in, no max-subtraction:
    |s*scale| <= ~4); double-buffered score psum so exp pipelines.
  - RoPE split across vector (lo half) and gpsimd (hi half) engines.
  - asum psum drained to SBUF by one gpsimd copy so the single accumulator
    psum buffer recycles fast; reciprocal via one fast custom-DVE op.
  - PSUM: scores 2x2 banks, accumulator 2, proj/outproj ring 2 = 8 exact.
  - All dram inputs pre-cast to bf16 on host (same RTNE rounding as chip).
"""

import os
import sys

sys.path.insert(0, "/opt/trn_rl_repo")

from contextlib import ExitStack

import numpy as np
import ml_dtypes

import concourse.bass as bass
import concourse.bacc as bacc
import concourse.tile as tile
import concourse.mybir as mybir
from concourse.bass import ts, ds
from concourse.bass_utils import run_bass_kernel_spmd

B, T, D, H, HD = 2, 2048, 1024, 16, 64
HPC = 4                # heads per core
E = HPC * HD           # 256 per-core channels
WP = 512               # projection chunk width (t)
WA = 256               # attention chunk width (q)
NPC = T // WP          # 4
NAC = T // WA          # 8
NKT = T // 128         # 16 k-tiles
DQ = D // 128          # 8 contraction subtiles
NEG = -10000.0
FP32 = mybir.dt.float32
BF16 = mybir.dt.bfloat16
SCALE = 1.0 / float(np.sqrt(HD))
NTT = T // 128         # 16 t-tiles for the output projection


def build_program(level=99):
    # level: debug truncation. 0=setup, 1=+prologue proj, 2=+chunk0 attn,
    # 3=+chunks<=3 w/ proj quanta, 4=+all chunks, 99=full (outproj+tail)
    nc = bacc.Bacc("TRN2", target_bir_lowering=False, debug=False)
    xT_in = nc.declare_dram_parameter("xT_b", [D, T], BF16, isOutput=False)
    wqT = nc.declare_dram_parameter("wqT", [D, E], BF16, isOutput=False)
    wkT = nc.declare_dram_parameter("wkT", [D, E], BF16, isOutput=False)
    wvT = nc.declare_dram_parameter("wvT", [D, E], BF16, isOutput=False)
    wpT = nc.declare_dram_parameter("wpT", [E, D], BF16, isOutput=False)
    cosT = nc.declare_dram_parameter("cosT", [128, T], BF16, isOutput=False)
    sinT = nc.declare_dram_parameter("sinT", [128, T], BF16, isOutput=False)
    umask = nc.declare_dram_parameter("umask", [128, 128], BF16, isOutput=False)
    lmask = nc.declare_dram_parameter("lmask", [128, 640], BF16, isOutput=False)
    outp = nc.declare_dram_parameter("outp", [T, D], FP32, isOutput=True)

    with tile.TileContext(nc) as tc, ExitStack() as ctx:
        consts = ctx.enter_context(tc.tile_pool(name="consts", bufs=1))
        ropet = ctx.enter_context(tc.tile_pool(name="ropet", bufs=2))
        probs_p = ctx.enter_context(
            tc.tile_pool(name="probs", bufs=int(os.environ.get("K_PRBUFS", "2")))
        )
        asb_p = ctx.enter_context(tc.tile_pool(name="asb", bufs=2))
        den_p = ctx.enter_context(tc.tile_pool(name="den", bufs=2))
        ostage = ctx.enter_context(tc.tile_pool(name="ostage", bufs=2))
        ps_sc = ctx.enter_context(
            tc.tile_pool(
                name="ps_sc", bufs=int(os.environ.get("K_SCBUFS", "2")), space="PSUM"
            )
        )
        ps_acc = ctx.enter_context(tc.tile_pool(name="ps_acc", bufs=1, space="PSUM"))
        ps_io = ctx.enter_context(tc.tile_pool(name="ps_io", bufs=1, space="PSUM"))

        # ---- constants / weights / x to SBUF ----
        # issue order matters: q-proj needs wq + xT first; cos/sin by rope
        # time; the diag mask only by the first attention chunk.
        xT_sb = consts.tile([128, DQ, T], BF16, tag="xT")
        xT_r = xT_in.rearrange("(o p) m -> p o m", p=128)
        wq_sb = consts.tile([128, DQ, E], BF16, tag="wq")
        nc.gpsimd.dma_start(wq_sb[:], wqT.rearrange("(o p) m -> p o m", p=128))
        for dq in range(DQ):
            eng = nc.gpsimd if dq % 2 == 0 else nc.sync
            eng.dma_start(xT_sb[:, dq, :], xT_r[:, dq, :])
        wk_sb = consts.tile([128, DQ, E], BF16, tag="wk")
        nc.gpsimd.dma_start(wk_sb[:], wkT.rearrange("(o p) m -> p o m", p=128))
        cos_sb = consts.tile([128, T], BF16, tag="cos")
        nc.scalar.dma_start(cos_sb[:], cosT[:])
        sin_sb = consts.tile([128, T], BF16, tag="sin")
        nc.scalar.dma_start(sin_sb[:], sinT[:])
        wv_sb = consts.tile([128, DQ, E], BF16, tag="wv")
        nc.gpsimd.dma_start(wv_sb[:], wvT.rearrange("(o p) m -> p o m", p=128))
        wp_sb = consts.tile([128, 2, D], BF16, tag="wp")
        nc.gpsimd.dma_start(wp_sb[:], wpT.rearrange("(o p) m -> p o m", p=128))
        u_sb = consts.tile([128, 128], BF16, tag="umask")
        nc.scalar.dma_start(u_sb[:], umask[:])
        lm_sb = consts.tile([128, 640], BF16, tag="lmask")
        nc.scalar.dma_start(lm_sb[:], lmask[:])

        # per-head q/k tiles on partitions 0:64 — keeps every scores matmul
        # at PE tile_position (0,0); base-64 matmuls closing a group on a
        # partial psum bank crash the runtime.
        q_nat = [
            consts.tile([64, T], BF16, tag=f"qnat{h}", name=f"qnat{h}")
            for h in range(HPC)
        ]
        k_nat = [
            consts.tile([64, T], BF16, tag=f"knat{h}", name=f"knat{h}")
            for h in range(HPC)
        ]
        # v_aug[:, t, h, :]: even h = [v | ones], odd h = [ones | v]; fill
        # everything with ones, the v copies overwrite their halves.
        v_aug = consts.tile([128, NKT, HPC, 128], BF16, tag="vaug")
        nc.gpsimd.memset(v_aug[:], 1.0)
        attn_nrm = [
            consts.tile([128, T], BF16, tag=f"anrm{p}", name=f"anrm{p}")
            for p in range(2)
        ]
        zer_sb = consts.tile([128, 128], BF16, tag="zer")
        nc.gpsimd.memset(zer_sb[:], 0.0)

        # ---- work quanta (proj / outproj), drained between attn iters ----
        def emit_qk(j, w_sb, nat):
            pqk = ps_io.tile([128, 2, WP], FP32, tag="io", name="pqk")
            for half in range(2):
                for dq in range(DQ):
                    nc.tensor.matmul(
                        pqk[:, half, :],
                        lhsT=w_sb[:, dq, ds(128 * half, 128)],
                        rhs=xT_sb[:, dq, ts(j, WP)],
                        start=(dq == 0),
                        stop=(dq == DQ - 1),
                    )
            lo, hi = pqk[:, 0, :], pqk[:, 1, :]
            cs, sn = cos_sb[:, ts(j, WP)], sin_sb[:, ts(j, WP)]
            st = ropet.tile([128, 2, WP], BF16, tag="st", name="st")
            ta = ropet.tile([128, 2, WP], FP32, tag="ta", name="ta")
            tb = ropet.tile([128, 2, WP], FP32, tag="tb", name="tb")
            nc.vector.tensor_mul(ta[:, 0, :], lo, cs)
            nc.vector.tensor_mul(ta[:, 1, :], hi, sn)
            nc.vector.tensor_sub(st[:, 0, :], ta[:, 0, :], ta[:, 1, :])
            nc.vector.tensor_mul(tb[:, 0, :], hi, cs)
            nc.vector.tensor_mul(tb[:, 1, :], lo, sn)
            nc.vector.tensor_add(st[:, 1, :], tb[:, 0, :], tb[:, 1, :])
            for h in range(HPC):
                nc.sync.dma_start(
                    nat[h][ds(0, 32), ts(j, WP)], st[ds(32 * h, 32), 0, :]
                )
                nc.sync.dma_start(
                    nat[h][ds(32, 32), ts(j, WP)], st[ds(32 * h, 32), 1, :]
                )

        def emit_v(j, half_pair):
            pv = ps_io.tile([128, 2, E], FP32, tag="io", name="pv")
            for tt in range(2):
                g = 4 * j + 2 * half_pair + tt
                for dq in range(DQ):
                    nc.tensor.matmul(
                        pv[:, tt, :],
                        lhsT=xT_sb[:, dq, ts(g, 128)],
                        rhs=wv_sb[:, dq, :],
                        start=(dq == 0),
                        stop=(dq == DQ - 1),
                    )
            for tt in range(2):
                g = 4 * j + 2 * half_pair + tt
                for h in range(HPC):
                    voff = 0 if h % 2 == 0 else 64
                    nc.vector.tensor_copy(
                        v_aug[:, g, h, ds(voff, 64)], pv[:, tt, ds(64 * h, 64)]
                    )

        def emit_po(g, pool=None, tag="io", tail=False):
            po = (pool or ps_io).tile([128, D], FP32, tag=tag, name="po")
            for dh in range(2):
                for p in range(2):
                    nc.tensor.matmul(
                        po[:, ds(512 * dh, 512)],
                        lhsT=attn_nrm[p][:, ts(g, 128)],
                        rhs=wp_sb[:, p, ds(512 * dh, 512)],
                        start=(p == 0),
                        stop=(p == 1),
                    )
            ost = ostage.tile([128, D], FP32, tag="ost", name="ost")
            if tail and g % 2 == 1:
                nc.scalar.copy(ost[:], po[:])
            else:
                nc.vector.tensor_copy(ost[:], po[:])
            if g % 2 == 0:
                nc.gpsimd.dma_start(outp[ts(g, 128), :], ost[:])
            else:
                nc.sync.dma_start(outp[ts(g, 128), :], ost[:])

        pending = []
        gap = [0]

        def drain_one():
            if pending and gap[0] >= 2:
                pending.pop(0)()
                gap[0] = 0

        # prologue: projection chunk 0 (serial; nothing to overlap with yet)
        if level >= 1 and not os.environ.get("K_NOPRO"):
            emit_qk(0, wq_sb, q_nat)
            emit_qk(0, wk_sb, k_nat)
            emit_v(0, 0)
            emit_v(0, 1)

        n_chunks = {0: 0, 1: 0, 2: 1, 3: 4, 4: NAC}.get(level, NAC)
        if os.environ.get("K_NCHUNKS"):
            n_chunks = int(os.environ["K_NCHUNKS"])
        a_start = int(os.environ.get("K_CHUNK_START", "0"))
        # ---- attention chunks, with quanta interleaved ----
        for a in range(a_start, n_chunks):
            if a % 2 == 0 and level >= 3 and not os.environ.get("K_NOQUANTA"):
                j = a // 2 + 1
                if j < NPC:
                    pending.append(lambda j=j: emit_qk(j, wq_sb, q_nat))
                    pending.append(lambda j=j: emit_qk(j, wk_sb, k_nat))
                    pending.append(lambda j=j: emit_v(j, 0))
                    pending.append(lambda j=j: emit_v(j, 1))
            if a >= 5 and level >= 5:
                # outproj for t-tiles, deferred late to fill bare iters
                po_sched = {5: range(0, 6), 6: range(6, 12), 7: range(12, 14)}
                for g in po_sched[a]:
                    pending.append(lambda g=g: emit_po(g))

            nk = 2 * a + 2
            if os.environ.get("K_MAXNK"):
                nk = min(nk, int(os.environ["K_MAXNK"]))
            asum = None
            if not os.environ.get("K_NOPV"):
                asum = ps_acc.tile([128, HPC, WA], FP32, tag="acc", name="asum")
            # start=True zeroes whole 2KB psum banks, so a per-head start
            # would wipe the co-banked head's accumulation; zero-init each
            # bank with one full-bank matmul instead.
            for bank in range(0 if not os.environ.get("K_NOPV") else -1, 2 if not os.environ.get("K_NOPV") else -1):
                nc.tensor.matmul(
                    asum[:, ds(2 * bank, 2), :],
                    lhsT=zer_sb[:],
                    rhs=xT_sb[:, 0, 0:512],
                    start=True,
                    stop=False,
                    skip_group_check=True,
                )

            def S(i, a=a):
                sct = ps_sc.tile([128, HPC, WA], FP32, tag="sc", name="sct")
                diag = i >= 2 * a
                for h in range(HPC):
                    nc.tensor.matmul(
                        sct[:, h, :],
                        lhsT=k_nat[h][:, ts(i, 128)],
                        rhs=q_nat[h][:, ts(a, WA)],
                        start=True,
                        stop=not diag,
                    )
                    if diag:
                        off = 384 - (128 * i - WA * a)
                        nc.tensor.matmul(
                            sct[:, h, :],
                            lhsT=u_sb[:],
                            rhs=lm_sb[:, ds(off, WA)],
                            start=False,
                            stop=True,
                        )
                return sct

            def EPV(i, sct, nk=nk, asum=asum, a=a):
                pr = probs_p.tile([128, HPC, WA], BF16, tag="pr", name="pr")
                nc.scalar.activation(
                    pr[:], sct[:], mybir.ActivationFunctionType.Exp, scale=SCALE
                )
                if os.environ.get("K_NOPV"):
                    return
                for h in range(HPC):
                    nc.tensor.matmul(
                        asum[:, h, :],
                        lhsT=v_aug[:, i, h, :],
                        rhs=pr[:, h, :],
                        start=False,
                        stop=(i == nk - 1),
                        skip_group_check=True,
                    )

            prev = None
            for i in range(nk):
                sct = S(i)
                if prev is not None:
                    EPV(prev[0], prev[1])
                prev = (i, sct)
                gap[0] += 1
                drain_one()
            EPV(prev[0], prev[1])

            if os.environ.get("K_NOEPI"):
                continue
            # epilogue: drain asum fast, then normalize off the critical path
            asb = asb_p.tile([128, HPC, WA], FP32, tag="asb", name="asb")
            nc.vector.tensor_copy(asb[:], asum[:])
            den = den_p.tile([128, 2, WA], FP32, tag="den", name="den")
            for p in range(2):
                nc.sync.dma_start(den[ds(0, 64), p, :], asb[ds(64, 64), 2 * p, :])
                nc.sync.dma_start(den[ds(64, 64), p, :], asb[ds(0, 64), 2 * p + 1, :])
            rc = den_p.tile([128, 2, WA], FP32, tag="rc", name="rc")
            nc.vector.reciprocal_approx_fast(rc[:], den[:])
            for p in range(2):
                nc.vector.tensor_mul(
                    attn_nrm[p][ds(0, 64), ts(a, WA)],
                    asb[ds(0, 64), 2 * p, :],
                    rc[ds(0, 64), p, :],
                )
                nc.vector.tensor_mul(
                    attn_nrm[p][ds(64, 64), ts(a, WA)],
                    asb[ds(64, 64), 2 * p + 1, :],
                    rc[ds(64, 64), p, :],
                )

        # tail: whatever quanta remain + last output tiles
        for f in pending:
            f()
        if level >= 5:
            # tail tiles go in the now-idle scores pool so they run in
            # parallel instead of serializing on the single-buffer io ring
            for g in range(NTT - 2, NTT):
                emit_po(g, pool=ps_sc, tag="sc", tail=True)

    nc.compile()
    return nc


def make_consts(cos, sin):
    cosT = np.ascontiguousarray(
        np.tile(np.asarray(cos[0], dtype=np.float32).T[:32], (4, 1))
    ).astype(ml_dtypes.bfloat16)
    sinT = np.ascontiguousarray(
        np.tile(np.asarray(sin[0], dtype=np.float32).T[:32], (4, 1))
    ).astype(ml_dtypes.bfloat16)
    m = np.arange(128)[:, None]
    r = np.arange(128)[None, :]
    umask = np.where(r >= m, NEG, 0.0).astype(ml_dtypes.bfloat16)
    u_idx = np.arange(640)[None, :]
    lmask = (m >= u_idx - 383).astype(ml_dtypes.bfloat16)
    return dict(cosT=cosT, sinT=sinT, umask=umask, lmask=lmask)


def host_prep(core, xT_by_batch, Wq, Wk, Wv, Wp, consts):
    b, hp = core // 4, core % 4
    h0 = hp * HPC
    rows = slice(HD * h0, HD * h0 + E)
    Wq_s = np.asarray(Wq[rows]).reshape(HPC, HD, D)
    Wk_s = np.asarray(Wk[rows]).reshape(HPC, HD, D)
    wqT = np.ascontiguousarray(
        np.concatenate(
            [Wq_s[:, :32].reshape(128, D), Wq_s[:, 32:].reshape(128, D)], 0
        ).T.astype(ml_dtypes.bfloat16)
    )
    wkT = np.ascontiguousarray(
        np.concatenate(
            [Wk_s[:, :32].reshape(128, D), Wk_s[:, 32:].reshape(128, D)], 0
        ).T.astype(ml_dtypes.bfloat16)
    )
    wvT = np.ascontiguousarray(np.asarray(Wv[rows]).T.astype(ml_dtypes.bfloat16))
    wpT = np.ascontiguousarray(np.asarray(Wp[:, rows]).T.astype(ml_dtypes.bfloat16))
    return dict(
        xT_b=xT_by_batch[b],
        wqT=wqT,
        wkT=wkT,
        wvT=wvT,
        wpT=wpT,
        **consts,
    )


_NC_CACHE = None


def _get_nc():
    global _NC_CACHE
    if _NC_CACHE is None:
        _NC_CACHE = build_program()
    return _NC_CACHE


def kernel(x, cos, sin, Wq, Wk, Wv, Wp, _want_trace=False):
    x, cos, sin = np.asarray(x), np.asarray(cos), np.asarray(sin)
    Wq, Wk, Wv, Wp = (np.asarray(a) for a in (Wq, Wk, Wv, Wp))
    nc = _get_nc()
    consts = make_consts(cos, sin)
    xT_by_batch = [
        np.ascontiguousarray(x[b].T.astype(ml_dtypes.bfloat16)) for b in range(B)
    ]
    in_maps = [
        host_prep(core, xT_by_batch, Wq, Wk, Wv, Wp, consts) for core in range(8)
    ]
    res = run_bass_kernel_spmd(nc, in_maps, list(range(8)), trace=_want_trace)
    out = np.zeros((B, T, D), dtype=np.float32)
    for core in range(8):
        out[core // 4] += np.asarray(res.results[core]["outp"], dtype=np.float32)
    if _want_trace:
        kernel.last_exec_time_ns = res.exec_time_ns
        kernel.last_profile = res.profile_json
    return out
